# revision 31
# baseline (speedup 1.0000x reference)
"""Causal MHA (B=4, S=2048, D=1024, H=16) on 8 NeuronCores.

Sharding: tensor-parallel over heads — 2 heads per core. Each core computes
Q/K/V projections for its 2 heads over the whole batch, causal flash-style
attention, and its partial output projection; the host sums the 8 partials.

Precision: the score path (x, Q/K/V projections, Q.K^T) runs in float32r
(tf32-like rounded fp32) on the PE array — 4x the matmul rate of fp32 for
~1.5e-4 matmul error, which the near-argmax softmax tolerates (measured
rel err ~9e-3 end to end vs the 2e-2 gate). Probs and o_proj are bf16.

Structure: per-block online softmax (block max + per-block exp from PSUM +
fused correction*1/l multiply) so PV directly yields normalized attn^T. The
causal mask on diagonal blocks is applied on the PE as a second matmul in the
score accumulation group (tri.T @ eye = staircase of -30720), keeping DVE out
of the mask path. Emission is software-pipelined across batches: batch b+1's
projections interleave with batch b's attention groups (fills the PE while
DVE/ACT drain softmax), and o_proj is emitted per-group right after each PV
so no serial tail forms at batch end. o_proj PSUM->SBUF copies alternate
between ACT and DVE (GPSIMD cannot access PSUM on HW).
"""

import numpy as np

B, S, D = 4, 2048, 1024
H, E = 16, 64
NCORES = 8
HPC = H // NCORES       # heads per core = 2
E2 = HPC * E            # 128
P = 128
KB = 512                # score block columns (k per psum tile)
SBLK = 512              # s-block for Q/K projection rhs tiles
NDT = D // P            # 8 d-tiles
NQT = S // P            # 16 q-tiles per batch
NST = S // P            # 16 s-tiles per batch
FMIN = -3.0e38

_BUILT = None


def _apply_drain_patch():
    """This walrus build rejects >1 sync-wait on a CTRL (Drain) instruction.
    Split the TileContext exit-drain waits across several drain instructions."""
    import concourse.tile as tile
    import concourse.mybir as mybir
    from concourse.vector_clock import ScopedClock

    if getattr(tile.TileContext, "_drain_patch_applied", False):
        return

    def _drain_and_barrier(self, tick_clock, wait_clock):
        nc = self.nc
        drain_inst = nc.sync.drain()
        wait_clock.add_sem_waits(
            drain_inst.ins, ScopedClock({None: tick_clock.global_clock})
        )
        si = drain_inst.ins.sync_info
        if si is not None and len(si.on_wait) > 1:
            waits = list(si.on_wait)
            del si.on_wait[1:]
            for w in waits[1:]:
                d2 = nc.sync.drain()
                d2.ins.sync_info = mybir.SyncInfo(on_wait=[w], on_update=[])
        nc.all_engine_barrier()
        popped = nc._tile_sem_poison_stack.pop()
        assert popped is self._sem_poison
        nc.clear_and_free_semaphores(list(self.sems.allocated().values()))
        nc.all_engine_barrier()

    tile.TileContext._drain_and_barrier = _drain_and_barrier
    tile.TileContext._drain_patch_applied = True


def _split_multiwaits(nc, max_waits=1):
    """This walrus build rejects instructions carrying more than ~1 sync-wait.
    Hoist extra waits onto single-wait NoOps on the same engine, placed just
    before the instruction (sequencers execute in order, so semantics hold)."""
    import concourse.mybir as mybir

    n_new = 0
    for f in nc.m.functions:
        for blk in f.blocks:
            insts = list(blk.instructions)
            if not any(
                getattr(i, "sync_info", None) is not None
                and len(i.sync_info.on_wait) > max_waits
                for i in insts
            ):
                continue
            out = []
            for inst in insts:
                si = getattr(inst, "sync_info", None)
                if si is not None and len(si.on_wait) > max_waits:
                    extra = list(si.on_wait[max_waits:])
                    del si.on_wait[max_waits:]
                    for w in extra:
                        n_new += 1
                        out.append(mybir.InstNoOp(
                            name=f"splitw-{n_new}",
                            sync_info=mybir.SyncInfo(on_wait=[w], on_update=[]),
                            engine=inst.engine,
                            bass_nofuse=True,
                        ))
                out.append(inst)
            blk.instructions[:] = out
    return n_new


def build_nc(reps=1):
    import concourse.bass as bass
    import concourse.mybir as mybir
    import concourse.tile as tile

    _apply_drain_patch()
    dt = mybir.dt
    Exp = mybir.ActivationFunctionType.Exp

    nc = bass.Bass("TRN2", target_bir_lowering=False, debug=False)

    x_d = nc.dram_tensor("x", [B, D, S], dt.float32r, kind="ExternalInput").ap()
    qw_d = nc.dram_tensor("qw", [D, E2], dt.float32r, kind="ExternalInput").ap()
    kw_d = nc.dram_tensor("kw", [D, E2], dt.float32r, kind="ExternalInput").ap()
    vw_d = nc.dram_tensor("vw", [D, E2], dt.float32r, kind="ExternalInput").ap()
    ow_d = nc.dram_tensor("ow", [E2, D], dt.bfloat16, kind="ExternalInput").ap()
    tri_d = nc.dram_tensor("tri", [P, P], dt.bfloat16, kind="ExternalInput").ap()
    eye_d = nc.dram_tensor("eye", [P, P], dt.bfloat16, kind="ExternalInput").ap()
    out_d = nc.dram_tensor("out", [B, S, D], dt.float32, kind="ExternalOutput").ap()

    with tile.TileContext(nc) as tc:
        with (
            tc.tile_pool(name="const", bufs=1) as cpool,
            tc.tile_pool(name="big", bufs=2) as big,
            tc.tile_pool(name="xs", bufs=2) as xs,
            tc.tile_pool(name="vt", bufs=3) as vt_p,
            tc.tile_pool(name="prow", bufs=6) as prow_p,
            tc.tile_pool(name="pt", bufs=1) as pt_p,
            tc.tile_pool(name="attnT", bufs=2) as attnT_p,
            tc.tile_pool(name="small", bufs=8) as small,
            tc.tile_pool(name="ost", bufs=6) as ost_p,
            # PSUM budget (8 banks): qko 2 + ss 4 + attnT 2
            tc.tile_pool(name="ps1", bufs=2, space="PSUM") as ps1,
            tc.tile_pool(name="psS", bufs=5, space="PSUM") as psS,
            tc.tile_pool(name="psa", bufs=1, space="PSUM") as psa_p,
        ):
            # ---- constants / weights ----
            qw_w = cpool.tile([P, NDT, E2], dt.float32r, tag="qw")
            nc.sync.dma_start(qw_w[:], qw_d.rearrange("(t p) e -> p t e", p=P))
            kw_w = cpool.tile([P, NDT, E2], dt.float32r, tag="kw")
            nc.sync.dma_start(kw_w[:], kw_d.rearrange("(t p) e -> p t e", p=P))
            vw_w = cpool.tile([P, NDT, E2], dt.float32r, tag="vw")
            nc.sync.dma_start(vw_w[:], vw_d.rearrange("(t p) e -> p t e", p=P))
            ow_w = cpool.tile([P, D], dt.bfloat16, tag="ow")
            nc.sync.dma_start(ow_w[:], ow_d)
            tri_sb = cpool.tile([P, P], dt.bfloat16, tag="tri")
            nc.sync.dma_start(tri_sb[:], tri_d)
            eye_sb = cpool.tile([P, P], dt.bfloat16, tag="eye")
            nc.sync.dma_start(eye_sb[:], eye_d)

            # ---- software-pipelined across batches: batch b's attention
            # interleaves batch b+1's projections (fills PE while DVE/ACT
            # drain softmax), and o_proj is emitted per-group right after
            # each PV so the batch tail stays short. ----
            proj_state = {}

            def start_proj(b):
                proj_state[b] = (
                    big.tile([P, S], dt.float32r, tag="QT", name=f"QT{b}"),
                    big.tile([P, S], dt.float32r, tag="KT", name=f"KT{b}"),
                    big.tile([P, NST, E2], dt.bfloat16, tag="Vs",
                             name=f"Vs{b}"),
                )

            def proj_step(b, sb):
                QT, KT, Vs = proj_state[b]
                ssl = slice(sb * SBLK, (sb + 1) * SBLK)
                x_t = xs.tile([P, NDT, SBLK], dt.float32r, tag="x")
                # halved loads so probs transposes can interleave sooner in
                # the DMA stream (big copies block the xbar-transpose path)
                xsrc = x_d[b, :, ssl].rearrange("(t p) s -> p t s", p=P)
                for t in range(0, NDT, 4):
                    nc.gpsimd.dma_start(x_t[:, t:t + 4], xsrc[:, t:t + 4])
                for dst, w_w in ((QT, qw_w), (KT, kw_w)):
                    ps = ps1.tile([P, SBLK], dt.float32, tag="qko", name="qk")
                    for t in range(NDT):
                        nc.tensor.matmul(
                            ps[:], w_w[:, t], x_t[:, t],
                            start=(t == 0), stop=(t == NDT - 1),
                        )
                    nc.scalar.copy(
                        out=dst[:, sb * SBLK:(sb + 1) * SBLK], in_=ps[:],
                    )
                # V^T chunk (bf16), then DMA-transpose into Vs
                psv = ps1.tile([P, SBLK], dt.float32, tag="qko", name="v")
                for t in range(NDT):
                    nc.tensor.matmul(
                        psv[:], vw_w[:, t], x_t[:, t],
                        start=(t == 0), stop=(t == NDT - 1),
                    )
                vt_t = vt_p.tile([P, SBLK], dt.bfloat16, tag="vt")
                nc.scalar.copy(out=vt_t[:], in_=psv[:])
                nc.sync.dma_start_transpose(
                    out=Vs[:, sb * (SBLK // P):(sb + 1) * (SBLK // P), :],
                    in_=vt_t[:],
                )

            start_proj(0)
            for sb in range(S // SBLK):
                proj_step(0, sb)

            for _rep, b in __import__('itertools').product(range(reps), range(B)):
                QT, KT, Vs = proj_state[b]
                if b + 1 < B:
                    start_proj(b + 1)

                # ---- attention for this batch ----
                attnT_sb = attnT_p.tile([P, NST, E2], dt.bfloat16, tag="attnT")

                def emit_pv(g, pt):
                    # PV for q-group g, heads col-packed; probs are normalized
                    # so this directly yields attnT. Per-j extents: q-col j only
                    # attends kt <= 4g+j, so no zero-fill of pt is needed.
                    psa = psa_p.tile([P, 4 * P], dt.float32, tag="a", name="a")
                    for j in range(4):
                        nkt = 4 * g + j + 1
                        for kt in range(nkt):
                            for h in range(HPC):
                                nc.tensor.matmul(
                                    psa[h * E:(h + 1) * E, j * P:(j + 1) * P],
                                    Vs[:, kt, h * E:(h + 1) * E],
                                    pt[h][:, kt, j * P:(j + 1) * P],
                                    start=(kt == 0), stop=(kt == nkt - 1),
                                )
                    nc.vector.tensor_copy(
                        out=attnT_sb[:, 4 * g:4 * (g + 1), :]
                            .rearrange("p a b -> p (a b)"),
                        in_=psa[:],
                    )
                    # o_proj for this group's 4 s-tiles right away: overlaps
                    # the rest of attention instead of serializing at the end
                    for st in range(4 * g, 4 * g + 4):
                        for dhalf in range(2):
                            pso = ps1.tile([P, 512], dt.float32, tag="qko",
                                           name="o")
                            nc.tensor.matmul(
                                pso[:], attnT_sb[:, st, :],
                                ow_w[:, dhalf * 512:(dhalf + 1) * 512],
                                start=True, stop=True,
                            )
                            osb = ost_p.tile([P, 512], dt.float32, tag="ost")
                            if dhalf == 0:
                                nc.scalar.copy(out=osb[:], in_=pso[:])
                            else:
                                nc.vector.tensor_copy(out=osb[:], in_=pso[:])
                            nc.sync.dma_start(
                                out_d[b, st * P:(st + 1) * P,
                                      dhalf * 512:(dhalf + 1) * 512],
                                osb[:],
                            )

                pending = None
                for g in range(NQT // 4):
                    kext = g + 1  # causal extent of the whole group, in KB blocks
                    pt = [pt_p.tile([P, NST, 4 * P], dt.bfloat16, tag=f"pt{h}",
                                    name=f"pt{h}") for h in range(HPC)]
                    for j in range(4):
                        qt = 4 * g + j
                        nmb = [small.tile([P, 4], dt.float32, tag=f"nmb{h}",
                                          name=f"nmb{h}") for h in range(HPC)]
                        lbuf = [small.tile([P, 4], dt.float32, tag=f"lb{h}",
                                           name=f"lb{h}") for h in range(HPC)]
                        prow = [prow_p.tile([P, S], dt.bfloat16, tag=f"prow{h}",
                                            name=f"prow{h}") for h in range(HPC)]
                        # valid columns in the last (diagonal) block
                        vext = (j + 1) * P
                        for kb in range(kext):
                            diag = kb == kext - 1
                            nv = vext if diag else KB
                            for h in range(HPC):
                                pss = psS.tile([P, KB], dt.float32, tag="ss")
                                nc.tensor.matmul(
                                    pss[:, :nv],
                                    QT[h * E:(h + 1) * E,
                                       qt * P:(qt + 1) * P],
                                    KT[h * E:(h + 1) * E,
                                       kb * KB:kb * KB + nv],
                                    start=True, stop=not diag,
                                )
                                if diag:
                                    # additive causal mask for the last 128
                                    # cols: mask[q,k'] = sum_c tri[c,q]*eye[c,k']
                                    nc.tensor.matmul(
                                        pss[:, j * P:(j + 1) * P],
                                        tri_sb[:], eye_sb[:],
                                        start=False, stop=True,
                                    )
                                nc.vector.reduce_max(
                                    out=nmb[h][:, kb:kb + 1], in_=pss[:, :nv],
                                    axis=mybir.AxisListType.X, negate=True,
                                )
                                nc.scalar.activation(
                                    out=prow[h][:, kb * KB:kb * KB + nv],
                                    in_=pss[:, :nv], func=Exp,
                                    bias=nmb[h][:, kb:kb + 1], scale=1.0,
                                    accum_out=lbuf[h][:, kb:kb + 1],
                                )
                        for h in range(HPC):
                            lr_h = small.tile([P, 1], dt.float32, tag=f"lr{h}",
                                              name=f"lr{h}")
                            if kext == 1:
                                nc.vector.reciprocal(lr_h[:], lbuf[h][:, 0:1])
                                nc.vector.tensor_scalar_mul(
                                    prow[h][:, :vext], prow[h][:, :vext], lr_h[:])
                            else:
                                negm = small.tile([P, 1], dt.float32,
                                                  tag=f"negm{h}", name=f"negm{h}")
                                cbuf = small.tile([P, 4], dt.float32,
                                                  tag=f"cb{h}", name=f"cb{h}")
                                l_h = small.tile([P, 1], dt.float32,
                                                 tag=f"l{h}", name=f"l{h}")
                                nc.vector.tensor_reduce(
                                    out=negm[:], in_=nmb[h][:, :kext],
                                    op=mybir.AluOpType.min,
                                    axis=mybir.AxisListType.X,
                                )
                                # c_kb = exp(m_kb - m) = exp(-nmb_kb + negm)
                                nc.scalar.activation(
                                    out=cbuf[:, :kext], in_=nmb[h][:, :kext],
                                    func=Exp, bias=negm[:], scale=-1.0,
                                )
                                nc.vector.tensor_tensor(
                                    lbuf[h][:, :kext], lbuf[h][:, :kext],
                                    cbuf[:, :kext], mybir.AluOpType.mult,
                                )
                                nc.vector.reduce_sum(
                                    out=l_h[:], in_=lbuf[h][:, :kext],
                                    axis=mybir.AxisListType.X,
                                )
                                nc.vector.reciprocal(lr_h[:], l_h[:])
                                # normalized correction: prow *= c_kb / l
                                nc.vector.tensor_scalar_mul(
                                    cbuf[:, :kext], cbuf[:, :kext], lr_h[:])
                                for kb in range(kext):
                                    nv = vext if kb == kext - 1 else KB
                                    nc.vector.tensor_scalar_mul(
                                        prow[h][:, kb * KB:kb * KB + nv],
                                        prow[h][:, kb * KB:kb * KB + nv],
                                        cbuf[:, kb:kb + 1],
                                    )
                            nc.sync.dma_start_transpose(
                                out=pt[h][:, :qt + 1, j * P:(j + 1) * P],
                                in_=prow[h][:, :(qt + 1) * P],
                            )
                        if j == 0 and pending is not None:
                            emit_pv(*pending)
                            pending = None
                    pending = (g, pt)
                    # interleave one projection block of the next batch
                    if b + 1 < B:
                        proj_step(b + 1, g)
                if pending is not None:
                    emit_pv(*pending)
                    pending = None
    _split_multiwaits(nc)
    return nc


def make_in_maps(in_feature, q_proj, k_proj, v_proj, o_proj):
    import ml_dtypes

    bf16 = ml_dtypes.bfloat16
    x = np.asarray(in_feature, np.float32)
    xT = np.ascontiguousarray(x.transpose(0, 2, 1))          # [B, D, S]

    scale = np.float32(1.0 / np.sqrt(E))
    qw = np.asarray(q_proj, np.float32).reshape(H, E, D) * scale
    kw = np.asarray(k_proj, np.float32).reshape(H, E, D)
    vw = np.asarray(v_proj, np.float32).reshape(H, E, D)
    ow = np.asarray(o_proj, np.float32).reshape(D, H, E)

    # causal mask building blocks: tri[c,q] = -30720 where c > q, eye = I.
    # mask matmul: (tri.T @ eye)[q,k'] = tri[k',q] = -30720*[k' > q]
    idx = np.arange(P)
    tri = np.where(idx[:, None] > idx[None, :], np.float32(-30720.0), 0.0)
    tri = tri.astype(bf16)
    eye = np.eye(P, dtype=np.float32).astype(bf16)

    in_maps = []
    for c in range(NCORES):
        sl = slice(HPC * c, HPC * (c + 1))
        qT = np.ascontiguousarray(qw[sl].reshape(E2, D).T)   # [D, E2]
        kT = np.ascontiguousarray(kw[sl].reshape(E2, D).T)
        vT = np.ascontiguousarray(vw[sl].reshape(E2, D).T)
        oT = np.ascontiguousarray(ow[:, sl, :].reshape(D, E2).T)  # [E2, D]
        in_maps.append({
            "x": xT,
            "qw": qT, "kw": kT,
            "vw": vT, "ow": oT.astype(bf16),
            "tri": tri, "eye": eye,
        })
    return in_maps


def kernel(in_feature, q_proj, k_proj, v_proj, o_proj, _results_hook=None):
    from concourse.bass_utils import run_bass_kernel_spmd

    global _BUILT
    if _BUILT is None:
        _BUILT = build_nc()
    in_maps = make_in_maps(in_feature, q_proj, k_proj, v_proj, o_proj)
    res = run_bass_kernel_spmd(_BUILT, in_maps, core_ids=list(range(NCORES)))
    if _results_hook is not None:
        _results_hook(res)
    out = np.zeros((B, S, D), np.float32)
    for r in res.results:
        out += r["out"]
    return out

